# revision 76
# baseline (speedup 1.0000x reference)
"""NT-Xent / SimCLR contrastive loss on 8 Trainium2 NeuronCores.

Problem: emb_i, emb_j [4096, 1024] f32 -> scalar loss.
  z = l2norm(rows); reps = concat(z_i, z_j) [2N, D]
  sim = reps @ reps.T;  loss = mean(-(pos/T - log(sum_offdiag exp(sim/T))))

Sharding (data parallel over the 2N=8192 rows, 1024 rows per core), with
the sim-matrix symmetry exploited so each [1024,1024] pair block is
computed once, not twice:

  - each core normalizes its 1024 local rows (scaled by S=64 so values
    sit in the fp8-e4m3 normal range) and transposes them to [D, rows]
    fp8,
  - the transposed matrix is AllGathered in TWO column-halves. The first
    half's collective triggers as soon as rows 0-511 are ready (~20us
    in); the second queues right behind it (queued collectives start
    ~1.7us after the previous one ends). While AG2 is on the wire the
    cores already compute half-1 foreign blocks (regular DMA queues and
    the collective CCOM queue rows are separate hardware, so g-loads
    legally overlap an in-flight collective),
  - symmetry: core c computes sim blocks only against peers c+1..c+4
    (mod 8). Blocks vs c+1..c+3 also produce exp-column-sums — DVE folds
    the 8 exp tiles into an f32 accumulator and one GpSimd
    partition_all_reduce collapses the partitions, so the colsum path
    never costs PE time; the c+4 block is computed by both endpoints (it
    covers the +-N positive diagonal). A 32KiB f32 ReduceScatter (row
    layout [8 targets, 1024], non-target rows zeroed, staged over the
    gpsimd SWDGE queue) then delivers each core the column-sum
    contributions from c-1..c-3. The received [1,1024] vector is
    transposed onto partitions with 8 tiny PE transposes and added to
    the local row-sums,
  - each [1024 x 512] half-block is 32 TensorE fp8 DoubleRow matmuls +
    one fused exp(2*sim)+row-sum ScalarE activation per (block, m,
    half); the self block is computed while the cores wait at the first
    collective's mesh barrier (launch skew makes that wait 30-70us),
  - positives come from a separate f32 path (host supplies each core's
    partner row block; loads + norms run inside the barrier-wait window);
    the self-diagonal term is exp(2), folded into the final Ln's bias,
  - per-row partial losses [128, 8] go back to the host, which sums and
    scales: a trivial gather.

Host-side work is only sharding/assembly: slicing rows, one np.eye, the
per-core offset table, and a final sum of the 8192 per-row losses.
"""

import math

import numpy as np
import ml_dtypes

import concourse.bacc as bacc
import concourse.bass as bass
import concourse.bass_isa as bass_isa
import concourse.mybir as mybir
import concourse.tile as tile
from concourse.bass_utils import run_bass_kernel_spmd

FP32 = mybir.dt.float32
BF16 = mybir.dt.bfloat16
FP8 = mybir.dt.float8e4
AF = mybir.ActivationFunctionType
ALU = mybir.AluOpType
PM = mybir.MatmulPerfMode

C = 8         # cores
N = 4096      # batch (per view)
D = 1024      # embedding dim
R = 1024      # local rows per core (2N / C)
P = 128       # partitions
MT = R // P   # m-tiles per core (8)
HC = 512      # columns per AllGather half
NT = 512      # PSUM bank free size (f32)
NFB = 4       # foreign blocks per core (peers c+1..c+4)
NSH = 3       # blocks whose column-sums are shared (peers c+1..c+3)
ESCALE = 2.0  # 1 / temperature
S = 64.0      # fp8 pre-quantization scale; exp scale folds in 1/S^2
LNS = math.log(S)
QSCALE = ESCALE / (S * S)
EDIAG = math.exp(ESCALE)  # self-sim diagonal term, exact to fp8 rounding
BLK_BYTES = P * MT * HC   # one core's half-block in ag_out_h, in ELEMENTS
ROW_OFF = R               # one row of the ReduceScatter table, in ELEMENTS


def _goff(c):
    g = np.zeros((1, C), dtype=np.uint32)
    for i in range(NFB):
        g[0, i] = ((c + 1 + i) % C) * BLK_BYTES
    for t in range(NSH):
        g[0, NFB + t] = ((c + 1 + t) % C) * ROW_OFF
    return g


def _build_kernel(tc, nc, xloc, xpart, ident, goff, out):
    with (
        tc.tile_pool(name="constp", bufs=1) as constp,
        tc.tile_pool(name="xmp", bufs=1) as xmp,      # 8 persistent local f32 tiles
        tc.tile_pool(name="pmp", bufs=1) as pmp,      # 8 persistent partner f32 tiles
        tc.tile_pool(name="zmp", bufs=1) as zmp,      # 8 persistent fp8 z tiles
        tc.tile_pool(name="ztp", bufs=1) as ztp,      # one [P, 2, MT, HC] fp8 zT tile
        tc.tile_pool(name="statp", bufs=1) as statp,
        tc.tile_pool(name="scrp", bufs=3) as scrp,    # [P, D] discard scratch
        tc.tile_pool(name="gp", bufs=1) as gp,        # 8 gathered fp8 half-blocks
        tc.tile_pool(name="psp", bufs=6, space="PSUM") as psp,   # [P,NT] 1 bank each
        tc.tile_pool(name="ptp", bufs=2, space="PSUM") as ptp,   # transpose staging
        tc.tile_pool(name="csap", bufs=2) as csap,    # [MT,NT] f32 colsum partials
        tc.tile_pool(name="expp", bufs=4) as expp,    # [P, NT] fp8 exp tiles
        tc.tile_pool(name="raccp", bufs=1) as raccp,
        tc.tile_pool(name="dramp", bufs=1, space="DRAM") as dramp,
    ):
        lns = statp.tile([P, 1], FP32, name="lns")
        nc.vector.memset(lns[:], LNS)
        nediag = statp.tile([P, 1], FP32, name="nediag")
        nc.vector.memset(nediag[:], -EDIAG)

        ss = statp.tile([P, MT], FP32, name="ss")
        ssp = statp.tile([P, MT], FP32, name="ssp")
        upos = statp.tile([P, MT], FP32, name="upos")
        rs = statp.tile([P, MT], FP32, name="rs")
        lss = statp.tile([P, MT], FP32, name="lss")

        # ---- phase 1: local row norms, scale, transpose to fp8 ----
        # Per-m pipeline: dma -> Square(accum) -> Ln -> Exp gives
        # rs_m = S/||x|| = exp(-.5*ln(ss)+ln(S)) (Rsqrt ACT is banned);
        # then DVE mul, PE transposes, DVE fp8 cast into the half-major
        # zt layout. After m=3 the first half is staged to DRAM with one
        # 128x4KiB-descriptor DMA and its AllGather triggers immediately;
        # the second half follows after m=7.
        ag_in = [dramp.tile([P, MT, HC], FP8, name=f"ag_in{h}")
                 for h in range(2)]
        ag_out = [dramp.tile([C * P, MT, HC], FP8, name=f"ag_out{h}",
                             addr_space="Shared") for h in range(2)]
        rs_in = dramp.tile([C, R], FP32, name="rs_in")
        rs_out = dramp.tile([1, R], FP32, name="rs_out")

        zrows = statp.tile([C, R], FP32, name="zrows")
        nc.vector.memset(zrows[:], 0.0)

        xms = []
        ccs = []
        identt = constp.tile([P, P], BF16, name="identt")
        gofft = constp.tile([1, C], mybir.dt.uint32, name="gofft")
        zt = ztp.tile([P, 2, MT, HC], FP8, name="zt")
        # input-tile DMAs issue per-m, interleaved with the norm chain and
        # alternating the two HWDGE rings. Batching all eight issues up
        # front was measured to cost ~5us on the first Square (the DMA
        # completion-semaphore thresholds batch across grouped issues);
        # interleaved issue keeps each tile's wait tight, and the first
        # collective's trigger time feeds 1:1 into the mesh barrier on
        # every core.
        for m in range(MT):
            xm = xmp.tile([P, D], FP32, name=f"xm{m}", tag=f"xm{m}")
            eng = nc.sync if m % 2 == 0 else nc.scalar
            eng.dma_start(xm[:], xloc[m * P:(m + 1) * P, :])
            xms.append(xm)
            if m == 1:
                # identt feeds the m0 transposes (~16us) — issue it right
                # after xm1 (32KB, negligible ring time)
                nc.scalar.dma_start(identt[:], ident[:])
            if m == MT - 1:
                # gofft/zrd issue last — their consumers (offset loads
                # ~50us, colsum staging ~115us) have huge slack
                nc.scalar.dma_start(gofft[:], goff[:])
                zrd = nc.scalar.dma_start(rs_in[:], zrows[:])
            # first-half norms split across ScalarE (even m: Square+accum)
            # and DVE (odd m: mul+reduce) so rs3 — the first collective's
            # gating value — is ready ~6us sooner than a serial ScalarE
            # chain; the zm scaling for those m's rides ScalarE's
            # per-partition activation scale to keep DVE clear
            if m < MT // 2 and m % 2 == 1:
                sqv = scrp.tile([P, D], FP32, name="sqv", tag="scr")
                nc.vector.tensor_mul(sqv[:], xm[:], xm[:])
                nc.vector.reduce_sum(ss[:, m:m + 1], sqv[:],
                                     axis=mybir.AxisListType.X)
            else:
                sq = scrp.tile([P, D], FP8, name="sq", tag="scr")
                nc.scalar.activation(sq[:], xm[:], AF.Square,
                                     accum_out=ss[:, m:m + 1])
            nc.scalar.activation(lss[:, m:m + 1], ss[:, m:m + 1], AF.Ln)
            nc.scalar.activation(rs[:, m:m + 1], lss[:, m:m + 1], AF.Exp,
                                 scale=-0.5, bias=lns[:])
            zm = zmp.tile([P, D], BF16, name=f"zm{m}", tag=f"zm{m}")
            if m < MT // 2:
                nc.scalar.activation(zm[:], xm[:], AF.Copy,
                                     scale=rs[:, m:m + 1])
            else:
                nc.vector.tensor_scalar_mul(zm[:], xm[:], rs[:, m:m + 1])
            pt = ptp.tile([P, MT, P], BF16, name="pt", tag="pt")
            for d in range(8):
                nc.tensor.matmul(pt[:, d, :], zm[:, d * P:(d + 1) * P],
                                 identt[:], is_transpose=True,
                                 skip_group_check=True)
            h, q = divmod(m, MT // 2)
            nc.vector.tensor_copy(zt[:, h, :, q * P:(q + 1) * P], pt[:])
            if m == MT // 2 - 1 or m == MT - 1:
                # ---- stage this half + AllGather it. The second cc queues
                # behind the first on the CC cores and starts ~1.7us after
                # it finishes; g-loads of half 1 overlap cc2's transfer.
                asm = nc.sync.dma_start(ag_in[h][:], zt[:, h, :, :])
                cc = nc.gpsimd.collective_compute(
                    "AllGather",
                    ALU.bypass,
                    replica_groups=[list(range(C))],
                    ins=[ag_in[h][:].opt()],
                    outs=[ag_out[h][:].opt()],
                )
                tile.add_dep_helper(cc.ins, asm.ins,
                                    reason="collective after its staging DMA")
                ccs.append(cc)

        # offset registers (4 g-block byte offsets + 3 ReduceScatter row
        # byte offsets), loaded while the SP queue idles under the
        # collectives
        offs = [
            nc.values_load(gofft[0:1, i:i + 1],
                           engines=[mybir.EngineType.SP if i < NFB
                                    else mybir.EngineType.Pool],
                           min_val=0, max_val=(C - 1) * BLK_BYTES,
                           skip_runtime_bounds_check=True)
            for i in range(NFB + NSH)
        ]

        # ---- phase 2a: self-similarity block, computed from the local zt
        # while the cores sit at the first collective's mesh barrier
        # (engine-only work, legal and free under a collective) ----
        racc = raccp.tile([P, MT, 2 * (NFB + 1)], FP32, name="racc")

        def half_block(m, h, rhs, acc_col, scalar_accum=False):
            ps = psp.tile([P, NT], FP32, name="ps", tag="ps")
            mh, q = divmod(m, MT // 2)
            for k in range(4):
                lhs = zt[:, mh, 2 * k:2 * k + 2, q * P:(q + 1) * P]
                nc.tensor.matmul(ps[:], lhs, rhs[:, 2 * k:2 * k + 2, 0:HC],
                                 start=(k == 0), stop=(k == 3),
                                 perf_mode=PM.DoubleRow)
            ed = expp.tile([P, NT], FP8, name="ed", tag="ed")
            if scalar_accum:
                # last wave slot: row-sum via the exp's own accumulator so
                # the preceding slot's colsum add stays the LAST DVE op of
                # the wave — its completion semaphore can't get batched
                # behind later DVE work, and the ReduceScatter triggers as
                # soon as that add lands
                nc.scalar.activation(ed[:], ps[:], AF.Exp, scale=QSCALE,
                                     accum_out=racc[:, m,
                                                    acc_col:acc_col + 1])
            else:
                # row-sum on DVE (idle during the waves): drops the 0.18us
                # ACTIVATION_READ_ACCUMULATOR from ScalarE's exp FIFO
                nc.scalar.activation(ed[:], ps[:], AF.Exp, scale=QSCALE)
                nc.vector.reduce_sum(racc[:, m, acc_col:acc_col + 1], ed[:],
                                     axis=mybir.AxisListType.X)
            return ed

        for m in range(MT):
            for h in range(2):
                half_block(m, h, zt[:, h], 2 * NFB + h)

        # ---- phase 1b: positives, entirely inside the mesh-barrier wait
        # window (the cores sit 30-70us at the first collective's barrier;
        # partner loads, DVE norms and the ScalarE tail all fit there, so
        # the wave phase sees a clean ScalarE FIFO of nothing but exps) ----
        pms = []

        def positives_dma_and_norms():
            for m in range(MT):
                pm = pmp.tile([P, D], FP32, name=f"pm{m}", tag=f"pm{m}")
                # ACT ring: keeps 4MiB of partner data off the SP ring that
                # feeds the g-block loads
                pmd = nc.scalar.dma_start(pm[:], xpart[m * P:(m + 1) * P, :])
                pms.append(pm)
            for m in range(MT):
                # (tensor_tensor_reduce faults this terminal's NRT with an
                # INTERNAL error — use plain mul + reduce instead)
                um = scrp.tile([P, D], FP32, name="um", tag="scr")
                nc.vector.tensor_mul(um[:], xms[m][:], pms[m][:])
                nc.vector.reduce_sum(upos[:, m:m + 1], um[:],
                                     axis=mybir.AxisListType.X)
                sqp = scrp.tile([P, D], FP32, name="sqp", tag="scr")
                nc.vector.tensor_mul(sqp[:], pms[m][:], pms[m][:])
                nc.vector.reduce_sum(ssp[:, m:m + 1], sqp[:],
                                     axis=mybir.AxisListType.X)

        def positives_tail():
            lssp = statp.tile([P, MT], FP32, name="lssp")
            nc.scalar.activation(lssp[:], ssp[:], AF.Ln)
            rsp = statp.tile([P, MT], FP32, name="rsp")
            nc.scalar.activation(rsp[:], lssp[:], AF.Exp, scale=-0.5,
                                 bias=lns[:])
            # pos2 = 2 * upos * (S/||x||) * (S/||p||) / S^2
            t1 = statp.tile([P, MT], FP32, name="t1")
            nc.vector.tensor_mul(t1[:], upos[:], rs[:])
            t2 = statp.tile([P, MT], FP32, name="t2")
            nc.vector.tensor_mul(t2[:], t1[:], rsp[:])
            pos2 = statp.tile([P, MT], FP32, name="pos2")
            nc.vector.tensor_scalar_mul(pos2[:], t2[:], QSCALE)
            return pos2

        positives_dma_and_norms()
        pos2 = positives_tail()

        # ---- phase 2b: the 4 foreign sim blocks (peers c+1..c+4), as two
        # waves of half-blocks. Wave h gates only on collective h, so wave
        # 0 runs while collective 1 is still on the wire. Blocks vs
        # c+1..c+3 (slots 0..2) also accumulate exp-column-sums for the
        # peer via ones-DoubleRow matmuls over m-paired ed tiles. ----
        cs_dmas = [zrd]
        csg = statp.tile([1, R], FP32, name="csg")
        # prefetch all 8 foreign half-blocks up front; each load gates only
        # on its own half's collective, so wave-1 blocks stream in while
        # wave 0 is still computing
        gs = []
        for h in range(2):
            row = []
            for i in range(NFB):
                g = gp.tile([P, MT, HC], FP8, name="g", tag=f"g{h}{i}")
                base = ag_out[h][0:P, :, :]
                src = bass.AP(tensor=base.tensor,
                              offset=base.offset + offs[i],
                              ap=base.ap, dep_tracking_offset=base.offset)
                gd = nc.sync.dma_start(g[:], src)
                tile.add_dep_helper(gd.ins, ccs[h].ins,
                                    reason="g-load after its half's collective")
                row.append(g)
            gs.append(row)
        # schedule: h-major (all of half 0, then all of half 1) measured
        # fastest: h1 slots never wait on AG2 (it lands mid-wave-0), and
        # shuffling shared slots earlier doesn't pay — the ReduceScatter
        # trigger lags its data by ~12us of Tile semaphore batching no
        # matter which engine consumes the colsums, so it fires near
        # wave-end regardless. The last slot keeps ScalarE row-sum
        # accumulation so no DVE work queues behind the gating colsum add.
        sched = [(0, 0), (1, 0), (2, 0), (3, 0),
                 (0, 1), (1, 1), (2, 1), (3, 1)]
        for pos, (i, h) in enumerate(sched):
                g = gs[h][i]
                share = i < NSH
                # column-sums without touching the PE: DVE folds the 8 exp
                # tiles into one f32 accumulator (it idles during the waves
                # now that positives run pre-barrier), then a single GpSimd
                # partition_all_reduce collapses the 128 partitions; the
                # staging DMA shares the gpsimd queue. This freed the ~13us
                # of PE wave time the old ones-matmul version spent.
                cs_acc = (csap.tile([P, NT], FP32, name="cs_acc", tag="csa")
                          if share else None)
                ed0 = None
                for m in range(MT):
                    ed = half_block(m, h, g[:], h * NFB + i,
                                    scalar_accum=(pos > 6))
                    if share:
                        if m == 0:
                            ed0 = ed
                        elif m == 1:
                            nc.vector.tensor_add(cs_acc[:], ed0[:], ed[:])
                        else:
                            nc.vector.tensor_add(cs_acc[:], cs_acc[:], ed[:])
                if share:
                    cs_sb = csap.tile([P, NT], FP32, name="cs_sb", tag="csr")
                    nc.gpsimd.partition_all_reduce(cs_sb[:], cs_acc[:], P,
                                                   bass_isa.ReduceOp.add)
                    rbase = rs_in[0:1, h * NT:(h + 1) * NT]
                    rdst = bass.AP(tensor=rbase.tensor,
                                   offset=rbase.offset + offs[NFB + i],
                                   ap=rbase.ap,
                                   dep_tracking_offset=rbase.offset)
                    csd = nc.gpsimd.dma_start(rdst, cs_sb[0:1, :])
                    tile.add_dep_helper(csd.ins, zrd.ins,
                                        reason="colsum rows after zero fill")
                    cs_dmas.append(csd)

        # ---- phase 3: exchange column-sums (32KiB f32 ReduceScatter),
        # transpose the received [1,1024] vector onto partitions, and
        # finish the per-row loss ----
        # local row-sum reduction emitted first so DVE folds racc while the
        # ReduceScatter is still on the wire
        rstot = statp.tile([P, MT], FP32, name="rstot")
        nc.vector.reduce_sum(rstot[:], racc[:], axis=mybir.AxisListType.X)

        cc3 = nc.gpsimd.collective_compute(
            "ReduceScatter",
            ALU.add,
            replica_groups=[list(range(C))],
            ins=[rs_in[:].opt()],
            outs=[rs_out[:].opt()],
        )
        for dma in cs_dmas:
            tile.add_dep_helper(cc3.ins, dma.ins,
                                reason="reduce-scatter after colsum staging")
        cgd = nc.sync.dma_start(csg[:], rs_out[:])
        tile.add_dep_helper(cgd.ins, cc3.ins,
                            reason="colsum gather after reduce-scatter")
        csgb = statp.tile([1, R], BF16, name="csgb")
        nc.vector.tensor_copy(csgb[:], csg[:])
        pst = ptp.tile([P, MT, P], BF16, name="pst", tag="pt")
        for m in range(MT):
            nc.tensor.matmul(pst[:, m, 0:1], csgb[0:1, m * P:(m + 1) * P],
                             identt[0:1, 0:1], is_transpose=True,
                             skip_group_check=True)
        csT = statp.tile([P, MT], FP32, name="csT")
        nc.vector.tensor_copy(csT[:], pst[:, :, 0])

        dsum = statp.tile([P, MT], FP32, name="dsum")
        logd = statp.tile([P, MT], FP32, name="logd")
        outv = statp.tile([P, MT], FP32, name="outv")
        nc.vector.tensor_add(dsum[:], rstot[:], csT[:])
        # the self-diagonal subtraction rides the Ln's bias port:
        # Ln(dsum - EDIAG), one DVE op + sync hop less on the RS-gated tail
        nc.scalar.activation(logd[:], dsum[:], AF.Ln, bias=nediag[:])
        nc.vector.tensor_sub(outv[:], logd[:], pos2[:])
        nc.sync.dma_start(out[:], outv[:])


_NC_CACHE = {}

COMBINED_ACT_SET = "natural_log_exp_and_others"


def _dedupe_act_table_loads(nc):
    """The stock act-table pass greedily loads the first set containing each
    activation function, thrashing ~1.5us table DMAs on every Ln<->Exp
    transition. Every function this kernel uses (Square/Ln/Exp/Copy) lives
    together in one set, so retarget every load to it and drop the
    now-redundant repeats. Bails out (no change) if an activation outside
    that set ever shows up."""
    import concourse.hw_specs as hw_specs
    tables = hw_specs.get_activation_tables(nc.m.arch)
    names = list(tables.keys())
    if COMBINED_ACT_SET not in names:
        return
    target = names.index(COMBINED_ACT_SET)
    covered = tables[COMBINED_ACT_SET]
    used = {
        ins.func
        for b in nc.main_func.blocks
        for ins in b.instructions
        if isinstance(ins, mybir.InstActivation)
    }
    if not used <= covered:
        return
    first_seen = False
    for b in nc.main_func.blocks:
        survivors = []
        for ins in b.instructions:
            if isinstance(ins, mybir.InstLoadActFuncSet):
                si = ins.sync_info
                has_sync = si is not None and (
                    len(si.on_wait) > 0 or len(si.on_update) > 0)
                ins.act_func_set_id = target
                if first_seen and not has_sync:
                    continue  # redundant reload of the same set
                first_seen = True
            survivors.append(ins)
        if len(survivors) != len(b.instructions):
            for idx in range(len(b.instructions) - 1, -1, -1):
                if b.instructions[idx] not in survivors:
                    del b.instructions[idx]


def build_nc():
    if "nc" in _NC_CACHE:
        return _NC_CACHE["nc"]
    nc = bacc.Bacc("TRN2", target_bir_lowering=False, debug=False,
                   num_devices=C)
    orig_iatl = nc.insert_act_table_loads

    def patched_iatl():
        orig_iatl()
        _dedupe_act_table_loads(nc)

    nc.insert_act_table_loads = patched_iatl
    xloc = nc.dram_tensor("xloc", [R, D], FP32, kind="ExternalInput")
    xpart = nc.dram_tensor("xpart", [R, D], FP32, kind="ExternalInput")
    ident = nc.dram_tensor("ident", [P, P], BF16, kind="ExternalInput")
    goff = nc.dram_tensor("goff", [1, C], mybir.dt.uint32,
                          kind="ExternalInput")
    out = nc.dram_tensor("out", [P, MT], FP32, kind="ExternalOutput")
    with tile.TileContext(nc) as tc:
        _build_kernel(tc, nc, xloc, xpart, ident, goff, out)
    nc.compile()
    _NC_CACHE["nc"] = nc
    return nc


def run(emb_i, emb_j, **spmd_kwargs):
    x = np.concatenate(
        [np.asarray(emb_i, dtype=np.float32),
         np.asarray(emb_j, dtype=np.float32)], axis=0)
    eye = np.eye(P, dtype=ml_dtypes.bfloat16)
    in_maps = []
    for c in range(C):
        p = (c + C // 2) % C
        in_maps.append({
            "xloc": np.ascontiguousarray(x[c * R:(c + 1) * R]),
            "xpart": np.ascontiguousarray(x[p * R:(p + 1) * R]),
            "ident": eye,
            "goff": _goff(c),
        })
    nc = build_nc()
    res = run_bass_kernel_spmd(nc, in_maps, core_ids=list(range(C)),
                               **spmd_kwargs)
    total = np.float64(0.0)
    for c in range(C):
        total += np.asarray(res.results[c]["out"], dtype=np.float64).sum()
    loss = np.float32(total / (2 * N))
    return loss, res


def kernel(emb_i, emb_j):
    loss, _ = run(emb_i, emb_j)
    return np.asarray(loss, dtype=np.float32)


# revision 77
# speedup vs baseline: 1.1169x; 1.1169x over previous
"""NT-Xent / SimCLR contrastive loss on 8 Trainium2 NeuronCores.

Problem: emb_i, emb_j [4096, 1024] f32 -> scalar loss.
  z = l2norm(rows); reps = concat(z_i, z_j) [2N, D]
  sim = reps @ reps.T;  loss = mean(-(pos/T - log(sum_offdiag exp(sim/T))))

Sharding (data parallel over the 2N=8192 rows, 1024 rows per core), with
the sim-matrix symmetry exploited so each [1024,1024] pair block is
computed once, not twice:

  - each core normalizes its 1024 local rows (scaled by S=64 so values
    sit in the fp8-e4m3 normal range) and transposes them to [D, rows]
    fp8,
  - the transposed matrix is AllGathered in TWO column-halves. The first
    half's collective triggers as soon as rows 0-511 are ready (~20us
    in); the second queues right behind it (queued collectives start
    ~1.7us after the previous one ends). While AG2 is on the wire the
    cores already compute half-1 foreign blocks (regular DMA queues and
    the collective CCOM queue rows are separate hardware, so g-loads
    legally overlap an in-flight collective),
  - symmetry: core c computes sim blocks only against peers c+1..c+4
    (mod 8). Blocks vs c+1..c+3 also produce exp-column-sums — DVE folds
    the 8 exp tiles into an f32 accumulator and one GpSimd
    partition_all_reduce collapses the partitions, so the colsum path
    never costs PE time; the c+4 block is computed by both endpoints (it
    covers the +-N positive diagonal). A 32KiB f32 ReduceScatter (row
    layout [8 targets, 1024], non-target rows zeroed, staged over the
    gpsimd SWDGE queue) then delivers each core the column-sum
    contributions from c-1..c-3. The received [1,1024] vector is
    transposed onto partitions with 8 tiny PE transposes and added to
    the local row-sums,
  - each [1024 x 512] half-block is 32 TensorE fp8 DoubleRow matmuls +
    one fused exp(2*sim)+row-sum ScalarE activation per (block, m,
    half); the self block is computed while the cores wait at the first
    collective's mesh barrier (launch skew makes that wait 30-70us),
  - positives come from a separate f32 path (host supplies each core's
    partner row block; loads + norms run inside the barrier-wait window);
    the self-diagonal term is exp(2), folded into the final Ln's bias,
  - per-row partial losses [128, 8] go back to the host, which sums and
    scales: a trivial gather.

Host-side work is only sharding/assembly: slicing rows, one np.eye, the
per-core offset table, and a final sum of the 8192 per-row losses.
"""

import math

import numpy as np
import ml_dtypes

import concourse.bacc as bacc
import concourse.bass as bass
import concourse.bass_isa as bass_isa
import concourse.mybir as mybir
import concourse.tile as tile
from concourse.bass_utils import run_bass_kernel_spmd

FP32 = mybir.dt.float32
BF16 = mybir.dt.bfloat16
FP8 = mybir.dt.float8e4
AF = mybir.ActivationFunctionType
ALU = mybir.AluOpType
PM = mybir.MatmulPerfMode

C = 8         # cores
N = 4096      # batch (per view)
D = 1024      # embedding dim
R = 1024      # local rows per core (2N / C)
P = 128       # partitions
MT = R // P   # m-tiles per core (8)
HC = 512      # columns per AllGather half
NT = 512      # PSUM bank free size (f32)
NFB = 4       # foreign blocks per core (peers c+1..c+4)
NSH = 3       # blocks whose column-sums are shared (peers c+1..c+3)
ESCALE = 2.0  # 1 / temperature
S = 64.0      # fp8 pre-quantization scale; exp scale folds in 1/S^2
LNS = math.log(S)
QSCALE = ESCALE / (S * S)
EDIAG = math.exp(ESCALE)  # self-sim diagonal term, exact to fp8 rounding
BLK_BYTES = P * MT * HC   # one core's half-block in ag_out_h, in ELEMENTS
ROW_OFF = R               # one row of the ReduceScatter table, in ELEMENTS


def _goff(c):
    g = np.zeros((1, C), dtype=np.uint32)
    for i in range(NFB):
        g[0, i] = ((c + 1 + i) % C) * BLK_BYTES
    for t in range(NSH):
        g[0, NFB + t] = ((c + 1 + t) % C) * ROW_OFF
    return g


def _build_kernel(tc, nc, xloc, xpart, ident, goff, out):
    with (
        tc.tile_pool(name="constp", bufs=1) as constp,
        tc.tile_pool(name="xmp", bufs=1) as xmp,      # 8 persistent local f32 tiles
        tc.tile_pool(name="pmp", bufs=1) as pmp,      # 8 persistent partner f32 tiles
        tc.tile_pool(name="zmp", bufs=1) as zmp,      # 8 persistent fp8 z tiles
        tc.tile_pool(name="ztp", bufs=1) as ztp,      # one [P, 2, MT, HC] fp8 zT tile
        tc.tile_pool(name="statp", bufs=1) as statp,
        tc.tile_pool(name="scrp", bufs=3) as scrp,    # [P, D] discard scratch
        tc.tile_pool(name="gp", bufs=1) as gp,        # 8 gathered fp8 half-blocks
        tc.tile_pool(name="psp", bufs=6, space="PSUM") as psp,   # [P,NT] 1 bank each
        tc.tile_pool(name="ptp", bufs=2, space="PSUM") as ptp,   # transpose staging
        tc.tile_pool(name="csap", bufs=2) as csap,    # [MT,NT] f32 colsum partials
        tc.tile_pool(name="expp", bufs=4) as expp,    # [P, NT] fp8 exp tiles
        tc.tile_pool(name="raccp", bufs=1) as raccp,
        tc.tile_pool(name="dramp", bufs=1, space="DRAM") as dramp,
    ):
        lns = statp.tile([P, 1], FP32, name="lns")
        nc.vector.memset(lns[:], LNS)
        nediag = statp.tile([P, 1], FP32, name="nediag")
        nc.vector.memset(nediag[:], -EDIAG)

        ss = statp.tile([P, MT], FP32, name="ss")
        ssp = statp.tile([P, MT], FP32, name="ssp")
        upos = statp.tile([P, MT], FP32, name="upos")
        rs = statp.tile([P, MT], FP32, name="rs")
        lss = statp.tile([P, MT], FP32, name="lss")

        # ---- phase 1: local row norms, scale, transpose to fp8 ----
        # Per-m pipeline: dma -> Square(accum) -> Ln -> Exp gives
        # rs_m = S/||x|| = exp(-.5*ln(ss)+ln(S)) (Rsqrt ACT is banned);
        # then DVE mul, PE transposes, DVE fp8 cast into the half-major
        # zt layout. After m=3 the first half is staged to DRAM with one
        # 128x4KiB-descriptor DMA and its AllGather triggers immediately;
        # the second half follows after m=7.
        ag_in = [dramp.tile([P, MT, HC], FP8, name=f"ag_in{h}")
                 for h in range(2)]
        ag_out = [dramp.tile([C * P, MT, HC], FP8, name=f"ag_out{h}",
                             addr_space="Shared") for h in range(2)]
        rs_in = dramp.tile([C, R], FP32, name="rs_in")
        rs_out = dramp.tile([1, R], FP32, name="rs_out")

        zrows = statp.tile([C, R], FP32, name="zrows")
        nc.vector.memset(zrows[:], 0.0)

        xms = []
        ccs = []
        identt = constp.tile([P, P], BF16, name="identt")
        gofft = constp.tile([1, C], mybir.dt.uint32, name="gofft")
        zt = ztp.tile([P, 2, MT, HC], FP8, name="zt")
        # all input-tile DMAs issue up front, alternating the two HWDGE
        # rings (SP / ACT) so tiles stream two at a time. (An interleaved
        # per-m issue variant measured a faster first collective trigger
        # but failed correctness intermittently — do not reintroduce it
        # without a long soak.)
        for m in list(range(MT // 2)) + list(range(MT // 2, MT)):
            xm = xmp.tile([P, D], FP32, name=f"xm{m}", tag=f"xm{m}")
            eng = nc.sync if m % 2 == 0 else nc.scalar
            eng.dma_start(xm[:], xloc[m * P:(m + 1) * P, :])
            xms.append(xm)
            if m == MT // 2 - 1:
                # constants slot in right after the half-1 tiles
                nc.scalar.dma_start(identt[:], ident[:])
                nc.scalar.dma_start(gofft[:], goff[:])
                zrd = nc.scalar.dma_start(rs_in[:], zrows[:])
        for m in range(MT):
            xm = xms[m]
            # first-half norms split across ScalarE (even m: Square+accum)
            # and DVE (odd m: mul+reduce) so rs3 — the first collective's
            # gating value — is ready ~6us sooner than a serial ScalarE
            # chain; the zm scaling for those m's rides ScalarE's
            # per-partition activation scale to keep DVE clear
            if m < MT // 2 and m % 2 == 1:
                sqv = scrp.tile([P, D], FP32, name="sqv", tag="scr")
                nc.vector.tensor_mul(sqv[:], xm[:], xm[:])
                nc.vector.reduce_sum(ss[:, m:m + 1], sqv[:],
                                     axis=mybir.AxisListType.X)
            else:
                sq = scrp.tile([P, D], FP8, name="sq", tag="scr")
                nc.scalar.activation(sq[:], xm[:], AF.Square,
                                     accum_out=ss[:, m:m + 1])
            nc.scalar.activation(lss[:, m:m + 1], ss[:, m:m + 1], AF.Ln)
            nc.scalar.activation(rs[:, m:m + 1], lss[:, m:m + 1], AF.Exp,
                                 scale=-0.5, bias=lns[:])
            zm = zmp.tile([P, D], BF16, name=f"zm{m}", tag=f"zm{m}")
            if m < MT // 2:
                nc.scalar.activation(zm[:], xm[:], AF.Copy,
                                     scale=rs[:, m:m + 1])
            else:
                nc.vector.tensor_scalar_mul(zm[:], xm[:], rs[:, m:m + 1])
            pt = ptp.tile([P, MT, P], BF16, name="pt", tag="pt")
            for d in range(8):
                nc.tensor.matmul(pt[:, d, :], zm[:, d * P:(d + 1) * P],
                                 identt[:], is_transpose=True,
                                 skip_group_check=True)
            h, q = divmod(m, MT // 2)
            nc.vector.tensor_copy(zt[:, h, :, q * P:(q + 1) * P], pt[:])
            if m == MT // 2 - 1 or m == MT - 1:
                # ---- stage this half + AllGather it. The second cc queues
                # behind the first on the CC cores and starts ~1.7us after
                # it finishes; g-loads of half 1 overlap cc2's transfer.
                asm = nc.sync.dma_start(ag_in[h][:], zt[:, h, :, :])
                cc = nc.gpsimd.collective_compute(
                    "AllGather",
                    ALU.bypass,
                    replica_groups=[list(range(C))],
                    ins=[ag_in[h][:].opt()],
                    outs=[ag_out[h][:].opt()],
                )
                tile.add_dep_helper(cc.ins, asm.ins,
                                    reason="collective after its staging DMA")
                ccs.append(cc)

        # offset registers (4 g-block byte offsets + 3 ReduceScatter row
        # byte offsets), loaded while the SP queue idles under the
        # collectives
        offs = [
            nc.values_load(gofft[0:1, i:i + 1],
                           engines=[mybir.EngineType.SP if i < NFB
                                    else mybir.EngineType.Pool],
                           min_val=0, max_val=(C - 1) * BLK_BYTES,
                           skip_runtime_bounds_check=True)
            for i in range(NFB + NSH)
        ]

        # ---- phase 2a: self-similarity block, computed from the local zt
        # while the cores sit at the first collective's mesh barrier
        # (engine-only work, legal and free under a collective) ----
        racc = raccp.tile([P, MT, 2 * (NFB + 1)], FP32, name="racc")

        def half_block(m, h, rhs, acc_col, scalar_accum=False):
            ps = psp.tile([P, NT], FP32, name="ps", tag="ps")
            mh, q = divmod(m, MT // 2)
            for k in range(4):
                lhs = zt[:, mh, 2 * k:2 * k + 2, q * P:(q + 1) * P]
                nc.tensor.matmul(ps[:], lhs, rhs[:, 2 * k:2 * k + 2, 0:HC],
                                 start=(k == 0), stop=(k == 3),
                                 perf_mode=PM.DoubleRow)
            ed = expp.tile([P, NT], FP8, name="ed", tag="ed")
            if scalar_accum:
                # last wave slot: row-sum via the exp's own accumulator so
                # the preceding slot's colsum add stays the LAST DVE op of
                # the wave — its completion semaphore can't get batched
                # behind later DVE work, and the ReduceScatter triggers as
                # soon as that add lands
                nc.scalar.activation(ed[:], ps[:], AF.Exp, scale=QSCALE,
                                     accum_out=racc[:, m,
                                                    acc_col:acc_col + 1])
            else:
                # row-sum on DVE (idle during the waves): drops the 0.18us
                # ACTIVATION_READ_ACCUMULATOR from ScalarE's exp FIFO
                nc.scalar.activation(ed[:], ps[:], AF.Exp, scale=QSCALE)
                nc.vector.reduce_sum(racc[:, m, acc_col:acc_col + 1], ed[:],
                                     axis=mybir.AxisListType.X)
            return ed

        for m in range(MT):
            for h in range(2):
                half_block(m, h, zt[:, h], 2 * NFB + h)

        # ---- phase 1b: positives, entirely inside the mesh-barrier wait
        # window (the cores sit 30-70us at the first collective's barrier;
        # partner loads, DVE norms and the ScalarE tail all fit there, so
        # the wave phase sees a clean ScalarE FIFO of nothing but exps) ----
        pms = []

        def positives_dma_and_norms():
            for m in range(MT):
                pm = pmp.tile([P, D], FP32, name=f"pm{m}", tag=f"pm{m}")
                # ACT ring: keeps 4MiB of partner data off the SP ring that
                # feeds the g-block loads
                pmd = nc.scalar.dma_start(pm[:], xpart[m * P:(m + 1) * P, :])
                pms.append(pm)
            for m in range(MT):
                # (tensor_tensor_reduce faults this terminal's NRT with an
                # INTERNAL error — use plain mul + reduce instead)
                um = scrp.tile([P, D], FP32, name="um", tag="scr")
                nc.vector.tensor_mul(um[:], xms[m][:], pms[m][:])
                nc.vector.reduce_sum(upos[:, m:m + 1], um[:],
                                     axis=mybir.AxisListType.X)
                sqp = scrp.tile([P, D], FP32, name="sqp", tag="scr")
                nc.vector.tensor_mul(sqp[:], pms[m][:], pms[m][:])
                nc.vector.reduce_sum(ssp[:, m:m + 1], sqp[:],
                                     axis=mybir.AxisListType.X)

        def positives_tail():
            lssp = statp.tile([P, MT], FP32, name="lssp")
            nc.scalar.activation(lssp[:], ssp[:], AF.Ln)
            rsp = statp.tile([P, MT], FP32, name="rsp")
            nc.scalar.activation(rsp[:], lssp[:], AF.Exp, scale=-0.5,
                                 bias=lns[:])
            # pos2 = 2 * upos * (S/||x||) * (S/||p||) / S^2
            t1 = statp.tile([P, MT], FP32, name="t1")
            nc.vector.tensor_mul(t1[:], upos[:], rs[:])
            t2 = statp.tile([P, MT], FP32, name="t2")
            nc.vector.tensor_mul(t2[:], t1[:], rsp[:])
            pos2 = statp.tile([P, MT], FP32, name="pos2")
            nc.vector.tensor_scalar_mul(pos2[:], t2[:], QSCALE)
            return pos2

        positives_dma_and_norms()
        pos2 = positives_tail()

        # ---- phase 2b: the 4 foreign sim blocks (peers c+1..c+4), as two
        # waves of half-blocks. Wave h gates only on collective h, so wave
        # 0 runs while collective 1 is still on the wire. Blocks vs
        # c+1..c+3 (slots 0..2) also accumulate exp-column-sums for the
        # peer via ones-DoubleRow matmuls over m-paired ed tiles. ----
        cs_dmas = [zrd]
        csg = statp.tile([1, R], FP32, name="csg")
        # prefetch all 8 foreign half-blocks up front; each load gates only
        # on its own half's collective, so wave-1 blocks stream in while
        # wave 0 is still computing
        gs = []
        for h in range(2):
            row = []
            for i in range(NFB):
                g = gp.tile([P, MT, HC], FP8, name="g", tag=f"g{h}{i}")
                base = ag_out[h][0:P, :, :]
                src = bass.AP(tensor=base.tensor,
                              offset=base.offset + offs[i],
                              ap=base.ap, dep_tracking_offset=base.offset)
                gd = nc.sync.dma_start(g[:], src)
                tile.add_dep_helper(gd.ins, ccs[h].ins,
                                    reason="g-load after its half's collective")
                row.append(g)
            gs.append(row)
        # schedule: h-major (all of half 0, then all of half 1) measured
        # fastest: h1 slots never wait on AG2 (it lands mid-wave-0), and
        # shuffling shared slots earlier doesn't pay — the ReduceScatter
        # trigger lags its data by ~12us of Tile semaphore batching no
        # matter which engine consumes the colsums, so it fires near
        # wave-end regardless. The last slot keeps ScalarE row-sum
        # accumulation so no DVE work queues behind the gating colsum add.
        sched = [(0, 0), (1, 0), (2, 0), (3, 0),
                 (0, 1), (1, 1), (2, 1), (3, 1)]
        for pos, (i, h) in enumerate(sched):
                g = gs[h][i]
                share = i < NSH
                # column-sums without touching the PE: DVE folds the 8 exp
                # tiles into one f32 accumulator (it idles during the waves
                # now that positives run pre-barrier), then a single GpSimd
                # partition_all_reduce collapses the 128 partitions; the
                # staging DMA shares the gpsimd queue. This freed the ~13us
                # of PE wave time the old ones-matmul version spent.
                cs_acc = (csap.tile([P, NT], FP32, name="cs_acc", tag="csa")
                          if share else None)
                ed0 = None
                for m in range(MT):
                    ed = half_block(m, h, g[:], h * NFB + i,
                                    scalar_accum=(pos > 6))
                    if share:
                        if m == 0:
                            ed0 = ed
                        elif m == 1:
                            nc.vector.tensor_add(cs_acc[:], ed0[:], ed[:])
                        else:
                            nc.vector.tensor_add(cs_acc[:], cs_acc[:], ed[:])
                if share:
                    cs_sb = csap.tile([P, NT], FP32, name="cs_sb", tag="csr")
                    nc.gpsimd.partition_all_reduce(cs_sb[:], cs_acc[:], P,
                                                   bass_isa.ReduceOp.add)
                    rbase = rs_in[0:1, h * NT:(h + 1) * NT]
                    rdst = bass.AP(tensor=rbase.tensor,
                                   offset=rbase.offset + offs[NFB + i],
                                   ap=rbase.ap,
                                   dep_tracking_offset=rbase.offset)
                    csd = nc.gpsimd.dma_start(rdst, cs_sb[0:1, :])
                    tile.add_dep_helper(csd.ins, zrd.ins,
                                        reason="colsum rows after zero fill")
                    cs_dmas.append(csd)

        # ---- phase 3: exchange column-sums (32KiB f32 ReduceScatter),
        # transpose the received [1,1024] vector onto partitions, and
        # finish the per-row loss ----
        # local row-sum reduction emitted first so DVE folds racc while the
        # ReduceScatter is still on the wire
        rstot = statp.tile([P, MT], FP32, name="rstot")
        nc.vector.reduce_sum(rstot[:], racc[:], axis=mybir.AxisListType.X)

        cc3 = nc.gpsimd.collective_compute(
            "ReduceScatter",
            ALU.add,
            replica_groups=[list(range(C))],
            ins=[rs_in[:].opt()],
            outs=[rs_out[:].opt()],
        )
        for dma in cs_dmas:
            tile.add_dep_helper(cc3.ins, dma.ins,
                                reason="reduce-scatter after colsum staging")
        cgd = nc.sync.dma_start(csg[:], rs_out[:])
        tile.add_dep_helper(cgd.ins, cc3.ins,
                            reason="colsum gather after reduce-scatter")
        csgb = statp.tile([1, R], BF16, name="csgb")
        nc.vector.tensor_copy(csgb[:], csg[:])
        pst = ptp.tile([P, MT, P], BF16, name="pst", tag="pt")
        for m in range(MT):
            nc.tensor.matmul(pst[:, m, 0:1], csgb[0:1, m * P:(m + 1) * P],
                             identt[0:1, 0:1], is_transpose=True,
                             skip_group_check=True)
        csT = statp.tile([P, MT], FP32, name="csT")
        nc.vector.tensor_copy(csT[:], pst[:, :, 0])

        dsum = statp.tile([P, MT], FP32, name="dsum")
        logd = statp.tile([P, MT], FP32, name="logd")
        outv = statp.tile([P, MT], FP32, name="outv")
        nc.vector.tensor_add(dsum[:], rstot[:], csT[:])
        # the self-diagonal subtraction rides the Ln's bias port:
        # Ln(dsum - EDIAG), one DVE op + sync hop less on the RS-gated tail
        nc.scalar.activation(logd[:], dsum[:], AF.Ln, bias=nediag[:])
        nc.vector.tensor_sub(outv[:], logd[:], pos2[:])
        nc.sync.dma_start(out[:], outv[:])


_NC_CACHE = {}

COMBINED_ACT_SET = "natural_log_exp_and_others"


def _dedupe_act_table_loads(nc):
    """The stock act-table pass greedily loads the first set containing each
    activation function, thrashing ~1.5us table DMAs on every Ln<->Exp
    transition. Every function this kernel uses (Square/Ln/Exp/Copy) lives
    together in one set, so retarget every load to it and drop the
    now-redundant repeats. Bails out (no change) if an activation outside
    that set ever shows up."""
    import concourse.hw_specs as hw_specs
    tables = hw_specs.get_activation_tables(nc.m.arch)
    names = list(tables.keys())
    if COMBINED_ACT_SET not in names:
        return
    target = names.index(COMBINED_ACT_SET)
    covered = tables[COMBINED_ACT_SET]
    used = {
        ins.func
        for b in nc.main_func.blocks
        for ins in b.instructions
        if isinstance(ins, mybir.InstActivation)
    }
    if not used <= covered:
        return
    first_seen = False
    for b in nc.main_func.blocks:
        survivors = []
        for ins in b.instructions:
            if isinstance(ins, mybir.InstLoadActFuncSet):
                si = ins.sync_info
                has_sync = si is not None and (
                    len(si.on_wait) > 0 or len(si.on_update) > 0)
                ins.act_func_set_id = target
                if first_seen and not has_sync:
                    continue  # redundant reload of the same set
                first_seen = True
            survivors.append(ins)
        if len(survivors) != len(b.instructions):
            for idx in range(len(b.instructions) - 1, -1, -1):
                if b.instructions[idx] not in survivors:
                    del b.instructions[idx]


def build_nc():
    if "nc" in _NC_CACHE:
        return _NC_CACHE["nc"]
    nc = bacc.Bacc("TRN2", target_bir_lowering=False, debug=False,
                   num_devices=C)
    orig_iatl = nc.insert_act_table_loads

    def patched_iatl():
        orig_iatl()
        _dedupe_act_table_loads(nc)

    nc.insert_act_table_loads = patched_iatl
    xloc = nc.dram_tensor("xloc", [R, D], FP32, kind="ExternalInput")
    xpart = nc.dram_tensor("xpart", [R, D], FP32, kind="ExternalInput")
    ident = nc.dram_tensor("ident", [P, P], BF16, kind="ExternalInput")
    goff = nc.dram_tensor("goff", [1, C], mybir.dt.uint32,
                          kind="ExternalInput")
    out = nc.dram_tensor("out", [P, MT], FP32, kind="ExternalOutput")
    with tile.TileContext(nc) as tc:
        _build_kernel(tc, nc, xloc, xpart, ident, goff, out)
    nc.compile()
    _NC_CACHE["nc"] = nc
    return nc


def run(emb_i, emb_j, **spmd_kwargs):
    x = np.concatenate(
        [np.asarray(emb_i, dtype=np.float32),
         np.asarray(emb_j, dtype=np.float32)], axis=0)
    eye = np.eye(P, dtype=ml_dtypes.bfloat16)
    in_maps = []
    for c in range(C):
        p = (c + C // 2) % C
        in_maps.append({
            "xloc": np.ascontiguousarray(x[c * R:(c + 1) * R]),
            "xpart": np.ascontiguousarray(x[p * R:(p + 1) * R]),
            "ident": eye,
            "goff": _goff(c),
        })
    nc = build_nc()
    res = run_bass_kernel_spmd(nc, in_maps, core_ids=list(range(C)),
                               **spmd_kwargs)
    total = np.float64(0.0)
    for c in range(C):
        total += np.asarray(res.results[c]["out"], dtype=np.float64).sum()
    loss = np.float32(total / (2 * N))
    return loss, res


def kernel(emb_i, emb_j):
    loss, _ = run(emb_i, emb_j)
    return np.asarray(loss, dtype=np.float32)
